# revision 4
# baseline (speedup 1.0000x reference)
"""BlockWiseHistogramEncoder Trainium2 kernel, v2 (digit-factorized PE design).

Input  x: [16, 1, 512, 512] int32, values in [0, 64).
Output:   [16, 1024, 65] float32. out[b, l, 1+v] = count(v in block l)/256,
out[b, l, 0] = 0. Blocks are 16x16, row-major (32x32 grid).

Sharding: pure data parallel over batch - 2 batches per core on 8 cores.

Per-core algorithm (2 batches, 2048 blocks of 256 elems):
  1. DMA-load block tiles [128 blocks, 256 elems] int32 (16 tiles).
  2. GPSIMD converts int32 -> int16.
  3. PE transposes each tile to element-major layout XT[eh][e128, l]
     (two 128x128 transposes per tile, int16 via identity matmul), DVE
     copies PSUM->SBUF.
  4. DVE builds digit masks at 4x: value v = 8*hi + lo;
     U_h[e,l] = ((v>>3)==h), V_d[e,l] = ((v&7)==d) as bf16 (16 masks).
  5. PE computes per-block joint counts: for each group of G=4 blocks,
     out[(h,l),(l',d)] = sum_e U[e,(h,l)] * V[e,(l',d)] via 2 PSUM-
     accumulated matmuls (e-halves). Groups are packed 4-per-PSUM-strip
     with tile_position column tiling, 16 supergroups per PSUM bank.
  6. ScalarE copies each PSUM bank to SBUF fp32 with scale 1/256.
  7. DVE extracts the l'==l diagonal with 4 partition-strided copies
     (partitions p = 32q+4h+l, so l = p%4; copy f-slice l'=lam from
     partitions p%4==lam).
  8. DMA scatters counts to y[b, l, 1+8h+d] (y pre-zeroed once).
"""
import sys

if "/opt/trn_rl_repo" not in sys.path:
    sys.path.insert(0, "/opt/trn_rl_repo")

import numpy as np

N_CORES = 8
B_PER_CORE = 2
H = W = 512
NC_CLS = 64
BLK = 16
HB = H // BLK          # 32 blocks per side
L = HB * HB            # 1024 blocks per batch
E = BLK * BLK          # 256 elems per block
TILES = L // 128       # 8 tiles of 128 blocks per batch

G = 4                  # blocks per matmul group (stationary = 32 cols)
NQ = 4                 # col-tiled groups per 32x128 PSUM strip set
SG = 16                # supergroups per PSUM bank (16*4*G = 256 blocks)
BLOCKS_PER_BANK = SG * NQ * G  # 256
BANKS_PER_BATCH = L // BLOCKS_PER_BANK  # 4

# within-32 partition permutation (q,h,l) -> (q,l,h):
# out partition i = 8l+h reads in partition 4h+l
SHUF = [4 * (i % 8) + i // 8 for i in range(32)]

_nc_cache = None
_run_cache = None


def _build(repeat: int = 1, hw_loop: bool = False, empty_loop: bool = False,
           stage: str = "full"):
    import concourse.bacc as bacc
    import concourse.mybir as mybir
    import concourse.tile as tile

    fp32 = mybir.dt.float32
    bf16 = mybir.dt.bfloat16
    i32 = mybir.dt.int32
    i16 = mybir.dt.int16

    nc = bacc.Bacc("TRN2", target_bir_lowering=False, debug=False)
    x = nc.dram_tensor("x_in", [B_PER_CORE, H, W], i32, kind="ExternalInput")
    # 64-wide (no zero class-0 col): keeps the out-scatter AP affine/3-dim.
    # Host prepends the constant zero column.
    y = nc.dram_tensor("y_out", [B_PER_CORE, L, NC_CLS], fp32,
                       kind="ExternalOutput")

    with tile.TileContext(nc) as tc:
        with tc.tile_pool(name="cst", bufs=1) as c_pool, \
             tc.tile_pool(name="io", bufs=4) as io_pool, \
             tc.tile_pool(name="cv", bufs=4) as cv_pool, \
             tc.tile_pool(name="xt", bufs=1) as xt_pool, \
             tc.tile_pool(name="mk", bufs=1) as mk_pool, \
             tc.tile_pool(name="ex", bufs=3) as ex_pool, \
             tc.tile_pool(name="ptr", bufs=4, space="PSUM") as ptr_pool, \
             tc.tile_pool(name="pmm", bufs=3, space="PSUM") as pmm_pool:

            # ---- constants ----
            # bf16 identity for PE transposes: iota(f - p) == 0
            ident = c_pool.tile([128, 128], bf16)
            iot = c_pool.tile([128, 128], i16, tag="iota")
            nc.gpsimd.iota(iot[:], pattern=[[1, 128]], base=0,
                           channel_multiplier=-1)
            nc.vector.tensor_scalar(
                ident[:], iot[:], 0, None, mybir.AluOpType.is_equal)

            # per-partition selector masks for the diagonal extraction:
            # p = 32q + 4h + l  ->  l(p) = p%4; msk[lam][p,:] = [l(p)==lam]
            pidx = c_pool.tile([128, 128], i32, tag="pidx")
            nc.gpsimd.iota(pidx[:], pattern=[[0, 128]], base=0,
                           channel_multiplier=1)
            lsel = c_pool.tile([128, 128], i32, tag="lsel")
            nc.vector.tensor_scalar(
                lsel[:], pidx[:], 3, None, mybir.AluOpType.bitwise_and)
            msk = []
            for lam in range(G):
                m = c_pool.tile([128, 128], i32, tag=f"msk{lam}",
                                name=f"msk{lam}")
                nc.vector.tensor_scalar(
                    m[:], lsel[:], lam, None, mybir.AluOpType.is_equal)
                msk.append(m)

            # persistent tensors, both batches merged along free dim
            L2 = B_PER_CORE * L
            XT = [xt_pool.tile([128, L2], i16, tag=f"xt{eh}",
                               name=f"xt{eh}") for eh in range(2)]
            # U interleaved: [e, group, h, l] so per-group weight slices
            # (h-major, l-minor) merge to one contiguous free dim; per-h
            # builds write packed (g, l) slices at 4x.
            U = [mk_pool.tile([128, L2 // G, 8, G], bf16, tag=f"u{eh}",
                              name=f"u{eh}") for eh in range(2)]
            # V interleaved: [e, group, d, l'] so per-group moving slices
            # (d-major, l'-minor) merge to one contiguous free dim, while
            # per-d mask builds still write packed (g, l') slices at 4x.
            V = [mk_pool.tile([128, L2 // G, 8, G], bf16, tag=f"v{eh}",
                              name=f"v{eh}") for eh in range(2)]

            xbs = [x.ap()[b].rearrange("(bh r) (bw c) -> bh bw r c",
                                       r=BLK, c=BLK)
                   for b in range(B_PER_CORE)]

            def load_tile(b, t):
                # [128 blocks, 256 elems] int32, strided DMA (64B chunks),
                # split across the SP and ACT hwdge queues
                t_in = io_pool.tile([128, E], i32)
                for i in range(4):
                    dst = t_in[32 * i:32 * (i + 1), :].rearrange(
                        "bw (r c) -> bw r c", c=BLK)
                    eng = nc.sync if i % 2 == 0 else nc.scalar
                    eng.dma_start(dst, xbs[b][4 * t + i])
                return t_in

            def convert_tile(t_in):
                # int32 -> bf16 (values < 64, exact)
                tb = cv_pool.tile([128, E], bf16)
                nc.gpsimd.tensor_copy(tb[:], t_in[:])
                return tb

            tr_state = {}

            def transpose_tile(b, t, tb):
                # two 128x128 bf16 transposes -> PSUM; copies PSUM->SBUF
                # (bf16 -> int16 convert) batched across 2 adjacent tiles
                tt = 2 * b * TILES + 2 * t  # global 2-col slot
                pair = tt // 4
                if pair not in tr_state:
                    tr_state[pair] = ptr_pool.tile([128, 512], bf16,
                                                   name="p_tr")
                p_tr = tr_state[pair]
                off = 256 * ((tt % 4) // 2)
                for eh in range(2):
                    nc.tensor.transpose(
                        p_tr[:, off + 128 * eh:off + 128 * (eh + 1)],
                        tb[:, 128 * eh:128 * (eh + 1)], ident[:])
                if off == 256 or (b == B_PER_CORE - 1 and t == TILES - 1):
                    del tr_state[pair]
                    col0 = 128 * (2 * pair)
                    n = 2 if off == 256 else 1
                    pv = p_tr[:].rearrange("e (u eh l) -> e u eh l",
                                           u=2, eh=2)
                    for eh in range(2):
                        nc.vector.tensor_copy(
                            XT[eh][:, col0:col0 + 128 * n],
                            pv[:, :n, eh, :])

            def build_masks(eh):
                xt = XT[eh]
                L2 = B_PER_CORE * L
                xh = cv_pool.tile([128, L2], i16, tag="xh", name="xh")
                nc.vector.tensor_scalar(
                    xh[:], xt[:], 56, None, mybir.AluOpType.bitwise_and)
                xl = cv_pool.tile([128, L2], i16, tag="xl", name="xl")
                nc.vector.tensor_scalar(
                    xl[:], xt[:], 7, None, mybir.AluOpType.bitwise_and)
                xhg = xh[:].rearrange("e (g l) -> e g l", l=G)
                xlg = xl[:].rearrange("e (g l) -> e g l", l=G)
                for h in range(8):
                    nc.vector.tensor_scalar(
                        U[eh][:, :, h, :], xhg, float(8 * h), None,
                        mybir.AluOpType.is_equal)
                for d in range(8):
                    nc.vector.tensor_scalar(
                        V[eh][:, :, d, :], xlg, float(d), None,
                        mybir.AluOpType.is_equal)

            def bank_groups(bank):
                # PE: fill one PSUM bank with SG supergroups x NQ col-strips
                pm = pmm_pool.tile([128, SG * 32], fp32)
                for sg in range(SG):
                    for q in range(NQ):
                        g0 = ((bank * SG + sg) * NQ + q) * G
                        out = pm[32 * q:32 * (q + 1), 32 * sg:32 * (sg + 1)]
                        for eh in range(2):
                            # lhsT: U [e128, (h8, l4)]; rhs: V [e128, (d8, l'4)]
                            # out[p=(h,l), f=(d,l')]; diag l'==l is useful
                            nc.tensor.matmul(
                                out, U[eh][:, g0 // G], V[eh][:, g0 // G],
                                start=(eh == 0), stop=(eh == 1),
                                tile_position=(0, 32 * q))
                return pm

            def extract_bank(gbank, pm):
                b, bank = divmod(gbank, BANKS_PER_BATCH)
                # ScalarE: PSUM fp32 -> SBUF fp32 with 1/256 scaling
                c_sb = ex_pool.tile([128, SG * 32], fp32, tag="c")
                nc.scalar.activation(
                    c_sb[:], pm[:], mybir.ActivationFunctionType.Copy,
                    bias=0.0, scale=1.0 / E)
                # DVE: diagonal extraction -- for each lam, copy f-slice
                # l'=lam into partitions with l(p)==lam (predicated).
                st = ex_pool.tile([128, SG * 8], fp32, tag="st")
                c_v = c_sb[:].rearrange("p (sg d l) -> p sg d l", l=G, d=8)
                for lam in range(G):
                    nc.vector.copy_predicated(
                        st[:], msk[lam][:, :SG * 8], c_v[:, :, :, lam])
                # permute partitions (q,h,l) -> (q,l,h) within each 32-group
                # so the out-scatter AP is affine (32B per partition step)
                stp = ex_pool.tile([128, SG * 8], fp32, tag="stp")
                nc.vector.stream_shuffle(stp[:], st[:], SHUF)
                # DMA: stp[p=(q,l,h), (sg,d)] -> y64[b, l_glob, 8h+d]
                # l_glob = ((bank*SG+sg)*NQ+q)*G + l; affine: p-stride 32B,
                # sg-stride 4096B, d-stride 4B.
                dst = y.ap()[b].rearrange(
                    "(bank sg q l) (h d) -> bank (q l h) sg d",
                    sg=SG, q=NQ, l=G, h=8)
                src = stp[:].rearrange("p (sg d) -> p sg d", d=8)
                nc.scalar.dma_start(dst[bank], src)

            # ---------------- pipeline ----------------
            def one_rep():
                for b in range(B_PER_CORE):
                    for t in range(TILES):
                        t_in = load_tile(b, t)
                        if stage == "ld":
                            continue
                        t16 = convert_tile(t_in)
                        if stage == "ldcv":
                            continue
                        transpose_tile(b, t, t16)
                if stage in ("ld", "ldcv", "ldtr"):
                    return
                for eh in range(2):
                    build_masks(eh)
                if stage == "msk":
                    return
                for gbank in range(2 * BANKS_PER_BATCH):
                    pm = bank_groups(gbank)
                    if stage == "mm":
                        continue
                    extract_bank(gbank, pm)

            if hw_loop:
                with tc.For_i(0, repeat):
                    if not empty_loop:
                        one_rep()
                    else:
                        nc.vector.memset(lsel[:, 0:1], 0.0)
                if empty_loop:
                    one_rep()
            else:
                for _rep in range(repeat):
                    one_rep()

    nc.compile()
    return nc


def _get_nc():
    global _nc_cache
    if _nc_cache is None:
        _nc_cache = _build()
    return _nc_cache


def _get_runner():
    """Build the sharded jitted executable once."""
    global _run_cache
    if _run_cache is not None:
        return _run_cache

    import jax
    from jax.sharding import Mesh, PartitionSpec
    from jax.experimental.shard_map import shard_map
    import concourse.mybir as mybir
    from concourse.bass2jax import (
        _bass_exec_p, install_neuronx_cc_hook, partition_id_tensor)

    nc = _get_nc()
    install_neuronx_cc_hook()

    partition_name = (nc.partition_id_tensor.name
                      if nc.partition_id_tensor else None)
    in_names, out_names, out_avals = [], [], []
    for alloc in nc.m.functions[0].allocations:
        if not isinstance(alloc, mybir.MemoryLocationSet):
            continue
        name = alloc.memorylocations[0].name
        if alloc.kind == "ExternalInput":
            if name != partition_name:
                in_names.append(name)
        elif alloc.kind == "ExternalOutput":
            out_names.append(name)
            out_avals.append(jax.core.ShapedArray(
                tuple(alloc.tensor_shape), mybir.dt.np(alloc.dtype)))
    n_params = len(in_names)
    n_outs = len(out_avals)
    all_in_names = list(in_names) + list(out_names)
    if partition_name is not None:
        all_in_names.append(partition_name)

    def _body(*args):
        operands = list(args)
        if partition_name is not None:
            operands.append(partition_id_tensor())
        outs = _bass_exec_p.bind(
            *operands,
            out_avals=tuple(out_avals),
            in_names=tuple(all_in_names),
            out_names=tuple(out_names),
            lowering_input_output_aliases=(),
            sim_require_finite=True,
            sim_require_nnan=True,
            nc=nc,
        )
        return tuple(outs)

    devices = jax.devices()[:N_CORES]
    mesh = Mesh(np.asarray(devices), ("core",))
    in_specs = (PartitionSpec("core"),) * (n_params + n_outs)
    out_specs = (PartitionSpec("core"),) * n_outs
    donate = tuple(range(n_params, n_params + n_outs))
    sharded = jax.jit(
        shard_map(_body, mesh=mesh, in_specs=in_specs, out_specs=out_specs,
                  check_rep=False),
        donate_argnums=donate, keep_unused=True)

    zero_shapes = [(N_CORES * a.shape[0], *a.shape[1:]) for a in out_avals]
    zero_dtypes = [a.dtype for a in out_avals]

    def run(concat_inputs):
        zeros = [np.zeros(s, d) for s, d in zip(zero_shapes, zero_dtypes)]
        out_arrs = sharded(*concat_inputs, *zeros)
        return {name: np.asarray(out_arrs[i]) for i, name in
                enumerate(out_names)}

    _run_cache = run
    return run


def kernel(x: np.ndarray) -> np.ndarray:
    assert x.shape == (16, 1, H, W) and x.dtype == np.int32, (x.shape, x.dtype)
    run = _get_runner()
    xs = np.ascontiguousarray(x[:, 0])          # [16, 512, 512]
    out = run([xs])["y_out"].reshape(16, L, NC_CLS)
    full = np.zeros((16, L, NC_CLS + 1), np.float32)
    full[:, :, 1:] = out
    return full


# revision 5
# speedup vs baseline: 1.0116x; 1.0116x over previous
"""BlockWiseHistogramEncoder Trainium2 kernel, v2 (digit-factorized PE design).

Input  x: [16, 1, 512, 512] int32, values in [0, 64).
Output:   [16, 1024, 65] float32. out[b, l, 1+v] = count(v in block l)/256,
out[b, l, 0] = 0. Blocks are 16x16, row-major (32x32 grid).

Sharding: pure data parallel over batch - 2 batches per core on 8 cores.

Per-core algorithm (2 batches, 2048 blocks of 256 elems):
  1. DMA-load block tiles [128 blocks, 256 elems] int32 (16 tiles).
  2. GPSIMD converts int32 -> int16.
  3. PE transposes each tile to element-major layout XT[eh][e128, l]
     (two 128x128 transposes per tile, int16 via identity matmul), DVE
     copies PSUM->SBUF.
  4. DVE builds digit masks at 4x: value v = 8*hi + lo;
     U_h[e,l] = ((v>>3)==h), V_d[e,l] = ((v&7)==d) as bf16 (16 masks).
  5. PE computes per-block joint counts: for each group of G=4 blocks,
     out[(h,l),(l',d)] = sum_e U[e,(h,l)] * V[e,(l',d)] via 2 PSUM-
     accumulated matmuls (e-halves). Groups are packed 4-per-PSUM-strip
     with tile_position column tiling, 16 supergroups per PSUM bank.
  6. ScalarE copies each PSUM bank to SBUF fp32 with scale 1/256.
  7. DVE extracts the l'==l diagonal with 4 partition-strided copies
     (partitions p = 32q+4h+l, so l = p%4; copy f-slice l'=lam from
     partitions p%4==lam).
  8. DMA scatters counts to y[b, l, 1+8h+d] (y pre-zeroed once).
"""
import sys

if "/opt/trn_rl_repo" not in sys.path:
    sys.path.insert(0, "/opt/trn_rl_repo")

import numpy as np

N_CORES = 8
B_PER_CORE = 2
H = W = 512
NC_CLS = 64
BLK = 16
HB = H // BLK          # 32 blocks per side
L = HB * HB            # 1024 blocks per batch
E = BLK * BLK          # 256 elems per block
TILES = L // 128       # 8 tiles of 128 blocks per batch

G = 4                  # blocks per matmul group (stationary = 8*G cols)

_nc_cache = None
_run_cache = None


def _build(repeat: int = 1, hw_loop: bool = False, empty_loop: bool = False,
           stage: str = "full", g: int = G):
    import concourse.bacc as bacc
    STRIP = 8 * g              # stationary cols per group
    NQ = 128 // STRIP          # col-tiled groups across partitions
    SG = 512 // STRIP          # supergroups (free slots) per PSUM bank
    NBANK = 2 * L // (SG * NQ * g)   # banks per core
    import concourse.mybir as mybir
    import concourse.tile as tile

    fp32 = mybir.dt.float32
    bf16 = mybir.dt.bfloat16
    i32 = mybir.dt.int32
    i16 = mybir.dt.int16

    nc = bacc.Bacc("TRN2", target_bir_lowering=False, debug=False)
    x = nc.dram_tensor("x_in", [B_PER_CORE, H, W], i32, kind="ExternalInput")
    # raw contiguous dump of the extraction staging; host unshuffles the
    # pure layout permutation and prepends the constant zero column
    y = nc.dram_tensor("y_out", [NBANK, 128, SG * 8], fp32,
                       kind="ExternalOutput")

    with tile.TileContext(nc) as tc:
        with tc.tile_pool(name="cst", bufs=1) as c_pool, \
             tc.tile_pool(name="io", bufs=4) as io_pool, \
             tc.tile_pool(name="cv", bufs=4) as cv_pool, \
             tc.tile_pool(name="xt", bufs=1) as xt_pool, \
             tc.tile_pool(name="mk", bufs=1) as mk_pool, \
             tc.tile_pool(name="ex", bufs=3) as ex_pool, \
             tc.tile_pool(name="ptr", bufs=4, space="PSUM") as ptr_pool, \
             tc.tile_pool(name="pmm", bufs=3, space="PSUM") as pmm_pool:

            # ---- constants ----
            # bf16 identity for PE transposes: iota(f - p) == 0
            ident = c_pool.tile([128, 128], bf16)
            iot = c_pool.tile([128, 128], i16, tag="iota")
            nc.gpsimd.iota(iot[:], pattern=[[1, 128]], base=0,
                           channel_multiplier=-1)
            nc.vector.tensor_scalar(
                ident[:], iot[:], 0, None, mybir.AluOpType.is_equal)

            # per-partition selector masks for the diagonal extraction:
            # p = STRIP*q + g*h + l  ->  l(p) = p%g; msk[lam] = [l(p)==lam]
            MW = SG * 8
            pidx = c_pool.tile([128, MW], i32, tag="pidx")
            nc.gpsimd.iota(pidx[:], pattern=[[0, MW]], base=0,
                           channel_multiplier=1)
            lsel = c_pool.tile([128, MW], i32, tag="lsel")
            nc.vector.tensor_scalar(
                lsel[:], pidx[:], g - 1, None, mybir.AluOpType.bitwise_and)
            msk = []
            for lam in range(g):
                m = c_pool.tile([128, MW], i32, tag=f"msk{lam}",
                                name=f"msk{lam}")
                nc.vector.tensor_scalar(
                    m[:], lsel[:], lam, None, mybir.AluOpType.is_equal)
                msk.append(m)

            # persistent tensors, both batches merged along free dim
            L2 = B_PER_CORE * L
            XT = [xt_pool.tile([128, L2], i16, tag=f"xt{eh}",
                               name=f"xt{eh}") for eh in range(2)]
            # U interleaved: [e, group, h, l] so per-group weight slices
            # (h-major, l-minor) merge to one contiguous free dim; per-h
            # builds write packed (g, l) slices at 4x.
            U = [mk_pool.tile([128, L2 // g, 8, g], bf16, tag=f"u{eh}",
                              name=f"u{eh}") for eh in range(2)]
            # V interleaved: [e, group, d, l'] so per-group moving slices
            # (d-major, l'-minor) merge to one contiguous free dim, while
            # per-d mask builds still write packed (g, l') slices at 4x.
            V = [mk_pool.tile([128, L2 // g, 8, g], bf16, tag=f"v{eh}",
                              name=f"v{eh}") for eh in range(2)]

            xbs = [x.ap()[b].rearrange("(bh r) (bw c) -> bh bw r c",
                                       r=BLK, c=BLK)
                   for b in range(B_PER_CORE)]

            def load_tile(b, t):
                # [128 blocks, 256 elems] int32, strided DMA (64B chunks),
                # split across the SP and ACT hwdge queues
                t_in = io_pool.tile([128, E], i32)
                for i in range(4):
                    dst = t_in[32 * i:32 * (i + 1), :].rearrange(
                        "bw (r c) -> bw r c", c=BLK)
                    eng = nc.sync if i % 2 == 0 else nc.scalar
                    eng.dma_start(dst, xbs[b][4 * t + i])
                return t_in

            def convert_tile(t_in):
                # int32 -> bf16 (values < 64, exact)
                tb = cv_pool.tile([128, E], bf16)
                nc.gpsimd.tensor_copy(tb[:], t_in[:])
                return tb

            tr_state = {}

            def transpose_tile(b, t, tb):
                # two 128x128 bf16 transposes -> PSUM; copies PSUM->SBUF
                # (bf16 -> int16 convert) batched across 2 adjacent tiles
                tt = 2 * b * TILES + 2 * t  # global 2-col slot
                pair = tt // 4
                if pair not in tr_state:
                    tr_state[pair] = ptr_pool.tile([128, 512], bf16,
                                                   name="p_tr")
                p_tr = tr_state[pair]
                off = 256 * ((tt % 4) // 2)
                for eh in range(2):
                    nc.tensor.transpose(
                        p_tr[:, off + 128 * eh:off + 128 * (eh + 1)],
                        tb[:, 128 * eh:128 * (eh + 1)], ident[:])
                if off == 256 or (b == B_PER_CORE - 1 and t == TILES - 1):
                    del tr_state[pair]
                    col0 = 128 * (2 * pair)
                    n = 2 if off == 256 else 1
                    pv = p_tr[:].rearrange("e (u eh l) -> e u eh l",
                                           u=2, eh=2)
                    for eh in range(2):
                        nc.vector.tensor_copy(
                            XT[eh][:, col0:col0 + 128 * n],
                            pv[:, :n, eh, :])

            def build_masks(eh):
                xt = XT[eh]
                L2 = B_PER_CORE * L
                xh = cv_pool.tile([128, L2], i16, tag="xh", name="xh")
                nc.vector.tensor_scalar(
                    xh[:], xt[:], 56, None, mybir.AluOpType.bitwise_and)
                xl = cv_pool.tile([128, L2], i16, tag="xl", name="xl")
                nc.vector.tensor_scalar(
                    xl[:], xt[:], 7, None, mybir.AluOpType.bitwise_and)
                xhg = xh[:].rearrange("e (g l) -> e g l", l=g)
                xlg = xl[:].rearrange("e (g l) -> e g l", l=g)
                for h in range(8):
                    nc.vector.tensor_scalar(
                        U[eh][:, :, h, :], xhg, float(8 * h), None,
                        mybir.AluOpType.is_equal)
                for d in range(8):
                    nc.vector.tensor_scalar(
                        V[eh][:, :, d, :], xlg, float(d), None,
                        mybir.AluOpType.is_equal)

            def bank_groups(bank):
                # PE: fill one PSUM bank with SG supergroups x NQ col-strips
                pm = pmm_pool.tile([128, 512], fp32)
                for sg in range(SG):
                    for q in range(NQ):
                        gi = (bank * SG + sg) * NQ + q
                        out = pm[STRIP * q:STRIP * (q + 1),
                                 STRIP * sg:STRIP * (sg + 1)]
                        for eh in range(2):
                            # lhsT: U [e128, (h8, l_g)]; rhs: V [e128, (d8, l'_g)]
                            # out[p=(h,l), f=(d,l')]; diag l'==l is useful
                            nc.tensor.matmul(
                                out, U[eh][:, gi], V[eh][:, gi],
                                start=(eh == 0), stop=(eh == 1),
                                tile_position=(0, STRIP * q))
                return pm

            def extract_bank(gbank, pm):
                # ScalarE: PSUM fp32 -> SBUF fp32 with 1/256 scaling
                c_sb = ex_pool.tile([128, 512], fp32, tag="c")
                nc.scalar.activation(
                    c_sb[:], pm[:], mybir.ActivationFunctionType.Copy,
                    bias=0.0, scale=1.0 / E)
                # DVE: diagonal extraction -- for each lam, copy f-slice
                # l'=lam into partitions with l(p)==lam (predicated).
                st = ex_pool.tile([128, SG * 8], fp32, tag="st")
                c_v = c_sb[:].rearrange("p (sg d l) -> p sg d l", l=g, d=8)
                for lam in range(g):
                    nc.vector.copy_predicated(
                        st[:], msk[lam][:], c_v[:, :, :, lam])
                # contiguous dump (512B runs); host unshuffles the layout
                nc.scalar.dma_start(y.ap()[gbank], st[:])

            # ---------------- pipeline ----------------
            def one_rep():
                for b in range(B_PER_CORE):
                    for t in range(TILES):
                        t_in = load_tile(b, t)
                        if stage == "ld":
                            continue
                        t16 = convert_tile(t_in)
                        if stage == "ldcv":
                            continue
                        transpose_tile(b, t, t16)
                if stage in ("ld", "ldcv", "ldtr"):
                    return
                for eh in range(2):
                    build_masks(eh)
                if stage == "msk":
                    return
                for gbank in range(NBANK):
                    pm = bank_groups(gbank)
                    if stage == "mm":
                        continue
                    extract_bank(gbank, pm)

            if hw_loop:
                with tc.For_i(0, repeat):
                    if not empty_loop:
                        one_rep()
                    else:
                        nc.vector.memset(lsel[:, 0:1], 0.0)
                if empty_loop:
                    one_rep()
            else:
                for _rep in range(repeat):
                    one_rep()

    nc.compile()
    return nc


def _get_nc():
    global _nc_cache
    if _nc_cache is None:
        _nc_cache = _build()
    return _nc_cache


def _get_runner():
    """Build the sharded jitted executable once."""
    global _run_cache
    if _run_cache is not None:
        return _run_cache

    import jax
    from jax.sharding import Mesh, PartitionSpec
    from jax.experimental.shard_map import shard_map
    import concourse.mybir as mybir
    from concourse.bass2jax import (
        _bass_exec_p, install_neuronx_cc_hook, partition_id_tensor)

    nc = _get_nc()
    install_neuronx_cc_hook()

    partition_name = (nc.partition_id_tensor.name
                      if nc.partition_id_tensor else None)
    in_names, out_names, out_avals = [], [], []
    for alloc in nc.m.functions[0].allocations:
        if not isinstance(alloc, mybir.MemoryLocationSet):
            continue
        name = alloc.memorylocations[0].name
        if alloc.kind == "ExternalInput":
            if name != partition_name:
                in_names.append(name)
        elif alloc.kind == "ExternalOutput":
            out_names.append(name)
            out_avals.append(jax.core.ShapedArray(
                tuple(alloc.tensor_shape), mybir.dt.np(alloc.dtype)))
    n_params = len(in_names)
    n_outs = len(out_avals)
    all_in_names = list(in_names) + list(out_names)
    if partition_name is not None:
        all_in_names.append(partition_name)

    def _body(*args):
        operands = list(args)
        if partition_name is not None:
            operands.append(partition_id_tensor())
        outs = _bass_exec_p.bind(
            *operands,
            out_avals=tuple(out_avals),
            in_names=tuple(all_in_names),
            out_names=tuple(out_names),
            lowering_input_output_aliases=(),
            sim_require_finite=True,
            sim_require_nnan=True,
            nc=nc,
        )
        return tuple(outs)

    devices = jax.devices()[:N_CORES]
    mesh = Mesh(np.asarray(devices), ("core",))
    in_specs = (PartitionSpec("core"),) * (n_params + n_outs)
    out_specs = (PartitionSpec("core"),) * n_outs
    donate = tuple(range(n_params, n_params + n_outs))
    sharded = jax.jit(
        shard_map(_body, mesh=mesh, in_specs=in_specs, out_specs=out_specs,
                  check_rep=False),
        donate_argnums=donate, keep_unused=True)

    zero_shapes = [(N_CORES * a.shape[0], *a.shape[1:]) for a in out_avals]
    zero_dtypes = [a.dtype for a in out_avals]

    def run(concat_inputs):
        zeros = [np.zeros(s, d) for s, d in zip(zero_shapes, zero_dtypes)]
        out_arrs = sharded(*concat_inputs, *zeros)
        return {name: np.asarray(out_arrs[i]) for i, name in
                enumerate(out_names)}

    _run_cache = run
    return run


def kernel(x: np.ndarray) -> np.ndarray:
    assert x.shape == (16, 1, H, W) and x.dtype == np.int32, (x.shape, x.dtype)
    run = _get_runner()
    xs = np.ascontiguousarray(x[:, 0])          # [16, 512, 512]
    out = run([xs])["y_out"]
    # device emits a contiguous per-bank dump; undo the layout permutation
    # (pure reshape/transpose) and prepend the constant zero column
    strip = 8 * G
    nq = 128 // strip
    sg = 512 // strip
    nbank = 2 * L // (sg * nq * G)
    r = out.reshape(N_CORES, nbank, nq, 8, G, sg, 8)  # c,bank,q,h,l,sg,d
    r = r.transpose(0, 1, 5, 2, 4, 3, 6)              # c,bank,sg,q,l,h,d
    r = r.reshape(16, L, NC_CLS)
    full = np.zeros((16, L, NC_CLS + 1), np.float32)
    full[:, :, 1:] = r
    return full


# revision 6
# speedup vs baseline: 1.1172x; 1.1044x over previous
"""BlockWiseHistogramEncoder Trainium2 kernel, v2 (digit-factorized PE design).

Input  x: [16, 1, 512, 512] int32, values in [0, 64).
Output:   [16, 1024, 65] float32. out[b, l, 1+v] = count(v in block l)/256,
out[b, l, 0] = 0. Blocks are 16x16, row-major (32x32 grid).

Sharding: pure data parallel over batch - 2 batches per core on 8 cores.

Per-core algorithm (2 batches, 2048 blocks of 256 elems):
  1. DMA-load block tiles [128 blocks, 256 elems] int32 (16 tiles).
  2. GPSIMD converts int32 -> int16.
  3. PE transposes each tile to element-major layout XT[eh][e128, l]
     (two 128x128 transposes per tile, int16 via identity matmul), DVE
     copies PSUM->SBUF.
  4. DVE builds digit masks at 4x: value v = 8*hi + lo;
     U_h[e,l] = ((v>>3)==h), V_d[e,l] = ((v&7)==d) as bf16 (16 masks).
  5. PE computes per-block joint counts: for each group of G=4 blocks,
     out[(h,l),(l',d)] = sum_e U[e,(h,l)] * V[e,(l',d)] via 2 PSUM-
     accumulated matmuls (e-halves). Groups are packed 4-per-PSUM-strip
     with tile_position column tiling, 16 supergroups per PSUM bank.
  6. ScalarE copies each PSUM bank to SBUF fp32 with scale 1/256.
  7. DVE extracts the l'==l diagonal with 4 partition-strided copies
     (partitions p = 32q+4h+l, so l = p%4; copy f-slice l'=lam from
     partitions p%4==lam).
  8. DMA scatters counts to y[b, l, 1+8h+d] (y pre-zeroed once).
"""
import sys

if "/opt/trn_rl_repo" not in sys.path:
    sys.path.insert(0, "/opt/trn_rl_repo")

import numpy as np

N_CORES = 8
B_PER_CORE = 2
H = W = 512
NC_CLS = 64
BLK = 16
HB = H // BLK          # 32 blocks per side
L = HB * HB            # 1024 blocks per batch
E = BLK * BLK          # 256 elems per block
TILES = L // 128       # 8 tiles of 128 blocks per batch

G = 4                  # blocks per matmul group (stationary = 8*G cols)

_nc_cache = None
_run_cache = None


def _build(repeat: int = 1, hw_loop: bool = False, empty_loop: bool = False,
           stage: str = "full", g: int = G):
    import concourse.bacc as bacc
    STRIP = 8 * g              # stationary cols per group
    NQ = 128 // STRIP          # col-tiled groups across partitions
    SG = 512 // STRIP          # supergroups (free slots) per PSUM bank
    NBANK = 2 * L // (SG * NQ * g)   # banks per core
    import concourse.mybir as mybir
    import concourse.tile as tile

    fp32 = mybir.dt.float32
    bf16 = mybir.dt.bfloat16
    i32 = mybir.dt.int32
    i16 = mybir.dt.int16

    nc = bacc.Bacc("TRN2", target_bir_lowering=False, debug=False)
    x = nc.dram_tensor("x_in", [B_PER_CORE, H, W], i32, kind="ExternalInput")
    # raw contiguous dump of the extraction staging; host unshuffles the
    # pure layout permutation and prepends the constant zero column
    y = nc.dram_tensor("y_out", [NBANK, 128, SG * 8], fp32,
                       kind="ExternalOutput")

    with tile.TileContext(nc) as tc:
        with tc.tile_pool(name="cst", bufs=1) as c_pool, \
             tc.tile_pool(name="io", bufs=4) as io_pool, \
             tc.tile_pool(name="cv", bufs=4) as cv_pool, \
             tc.tile_pool(name="xt", bufs=1) as xt_pool, \
             tc.tile_pool(name="mk", bufs=1) as mk_pool, \
             tc.tile_pool(name="ex", bufs=3) as ex_pool, \
             tc.tile_pool(name="ptr", bufs=4, space="PSUM") as ptr_pool, \
             tc.tile_pool(name="pmm", bufs=3, space="PSUM") as pmm_pool:

            # ---- constants ----
            # bf16 identity for PE transposes: iota(f - p) == 0
            ident = c_pool.tile([128, 128], bf16)
            iot = c_pool.tile([128, 128], i16, tag="iota")
            nc.gpsimd.iota(iot[:], pattern=[[1, 128]], base=0,
                           channel_multiplier=-1)
            nc.vector.tensor_scalar(
                ident[:], iot[:], 0, None, mybir.AluOpType.is_equal)

            # per-partition selector masks for the diagonal extraction:
            # p = STRIP*q + g*h + l  ->  l(p) = p%g; msk[lam] = [l(p)==lam]
            MW = SG * 8
            pidx = c_pool.tile([128, MW], i32, tag="pidx")
            nc.gpsimd.iota(pidx[:], pattern=[[0, MW]], base=0,
                           channel_multiplier=1)
            lsel = c_pool.tile([128, MW], i32, tag="lsel")
            nc.vector.tensor_scalar(
                lsel[:], pidx[:], g - 1, None, mybir.AluOpType.bitwise_and)
            msk = []
            for lam in range(g):
                m = c_pool.tile([128, MW], i32, tag=f"msk{lam}",
                                name=f"msk{lam}")
                nc.vector.tensor_scalar(
                    m[:], lsel[:], lam, None, mybir.AluOpType.is_equal)
                msk.append(m)

            # persistent per-batch tensors
            XT = [[xt_pool.tile([128, L], i16, tag=f"xt{b}{eh}",
                                name=f"xt{b}{eh}") for eh in range(2)]
                  for b in range(B_PER_CORE)]
            # U interleaved: [e, group, h, l] so per-group weight slices
            # (h-major, l-minor) merge to one contiguous free dim; per-h
            # builds write packed (g, l) slices at 4x.
            U = [[mk_pool.tile([128, L // g, 8, g], bf16, tag=f"u{b}{eh}",
                               name=f"u{b}{eh}") for eh in range(2)]
                 for b in range(B_PER_CORE)]
            # V interleaved: [e, group, d, l'] so per-group moving slices
            # (d-major, l'-minor) merge to one contiguous free dim, while
            # per-d mask builds still write packed (g, l') slices at 4x.
            V = [[mk_pool.tile([128, L // g, 8, g], bf16, tag=f"v{b}{eh}",
                               name=f"v{b}{eh}") for eh in range(2)]
                 for b in range(B_PER_CORE)]

            xbs = [x.ap()[b].rearrange("(bh r) (bw c) -> bh bw r c",
                                       r=BLK, c=BLK)
                   for b in range(B_PER_CORE)]

            def load_tile(b, t):
                # [128 blocks, 256 elems] int32, strided DMA (64B chunks),
                # split across the SP and ACT hwdge queues
                t_in = io_pool.tile([128, E], i32)
                for i in range(4):
                    dst = t_in[32 * i:32 * (i + 1), :].rearrange(
                        "bw (r c) -> bw r c", c=BLK)
                    eng = nc.sync if i % 2 == 0 else nc.scalar
                    eng.dma_start(dst, xbs[b][4 * t + i])
                return t_in

            def convert_tile(t_in):
                # int32 -> bf16 (values < 64, exact)
                tb = cv_pool.tile([128, E], bf16)
                nc.gpsimd.tensor_copy(tb[:], t_in[:])
                return tb

            tr_state = {}

            def transpose_tile(b, t, tb):
                # two 128x128 bf16 transposes -> PSUM; copies PSUM->SBUF
                # (bf16 -> int16 convert) batched across 2 adjacent tiles
                pair = t // 2
                if pair not in tr_state:
                    tr_state[pair] = ptr_pool.tile([128, 512], bf16,
                                                   name="p_tr")
                p_tr = tr_state[pair]
                off = 256 * (t % 2)
                for eh in range(2):
                    nc.tensor.transpose(
                        p_tr[:, off + 128 * eh:off + 128 * (eh + 1)],
                        tb[:, 128 * eh:128 * (eh + 1)], ident[:])
                if off == 256:
                    del tr_state[pair]
                    col0 = 256 * pair
                    pv = p_tr[:].rearrange("e (u eh l) -> e u eh l",
                                           u=2, eh=2)
                    for eh in range(2):
                        nc.vector.tensor_copy(
                            XT[b][eh][:, col0:col0 + 256],
                            pv[:, :, eh, :])

            def build_masks(b, eh):
                xt = XT[b][eh]
                xh = cv_pool.tile([128, L], i16, tag="xh", name="xh")
                nc.vector.tensor_scalar(
                    xh[:], xt[:], 56, None, mybir.AluOpType.bitwise_and)
                xl = cv_pool.tile([128, L], i16, tag="xl", name="xl")
                nc.vector.tensor_scalar(
                    xl[:], xt[:], 7, None, mybir.AluOpType.bitwise_and)
                xhg = xh[:].rearrange("e (g l) -> e g l", l=g)
                xlg = xl[:].rearrange("e (g l) -> e g l", l=g)
                for h in range(8):
                    nc.vector.tensor_scalar(
                        U[b][eh][:, :, h, :], xhg, float(8 * h), None,
                        mybir.AluOpType.is_equal)
                for d in range(8):
                    nc.vector.tensor_scalar(
                        V[b][eh][:, :, d, :], xlg, float(d), None,
                        mybir.AluOpType.is_equal)

            def bank_groups(b, bank):
                # PE: fill one PSUM bank with SG supergroups x NQ col-strips
                pm = pmm_pool.tile([128, 512], fp32)
                # interleave (eh, q) within each sg so consecutive matmuls
                # never target the same col-strip: the PE can pull the next
                # LDWEIGHTS ahead of the in-flight matmul (different col_grp)
                for sg in range(SG):
                    for eh in range(2):
                        for q in range(NQ):
                            gi = (bank * SG + sg) * NQ + q
                            out = pm[STRIP * q:STRIP * (q + 1),
                                     STRIP * sg:STRIP * (sg + 1)]
                            # lhsT: U [e128, (h8, l_g)]; rhs: V [e128, (d8, l'_g)]
                            # out[p=(h,l), f=(d,l')]; diag l'==l is useful
                            nc.tensor.matmul(
                                out, U[b][eh][:, gi], V[b][eh][:, gi],
                                start=(eh == 0), stop=(eh == 1),
                                tile_position=(0, STRIP * q))
                return pm

            def extract_bank(gbank, pm):
                # ScalarE: PSUM fp32 -> SBUF fp32 with 1/256 scaling
                c_sb = ex_pool.tile([128, 512], fp32, tag="c")
                nc.scalar.activation(
                    c_sb[:], pm[:], mybir.ActivationFunctionType.Copy,
                    bias=0.0, scale=1.0 / E)
                # DVE: diagonal extraction -- for each lam, copy f-slice
                # l'=lam into partitions with l(p)==lam (predicated).
                st = ex_pool.tile([128, SG * 8], fp32, tag="st")
                c_v = c_sb[:].rearrange("p (sg d l) -> p sg d l", l=g, d=8)
                for lam in range(g):
                    nc.vector.copy_predicated(
                        st[:], msk[lam][:], c_v[:, :, :, lam])
                # contiguous dump (512B runs); host unshuffles the layout
                nc.sync.dma_start(y.ap()[gbank], st[:])

            # ---------------- pipeline ----------------
            def front(b):
                for t in range(TILES):
                    t_in = load_tile(b, t)
                    if stage == "ld":
                        continue
                    t16 = convert_tile(t_in)
                    if stage == "ldcv":
                        continue
                    transpose_tile(b, t, t16)

            def mid(b):
                if stage in ("ld", "ldcv", "ldtr"):
                    return
                for eh in range(2):
                    build_masks(b, eh)

            def back(b):
                if stage in ("ld", "ldcv", "ldtr", "msk"):
                    return
                nb = NBANK // B_PER_CORE
                for bank in range(nb):
                    pm = bank_groups(b, bank)
                    if stage == "mm":
                        continue
                    extract_bank(b * nb + bank, pm)

            def one_rep():
                # emit b1's loads between b0's masks and b0's banks so the
                # ACT-queue load issues overlap b0 compute, not extraction
                front(0)
                mid(0)
                front(1)
                back(0)
                mid(1)
                back(1)

            if hw_loop:
                with tc.For_i(0, repeat):
                    if not empty_loop:
                        one_rep()
                    else:
                        nc.vector.memset(lsel[:, 0:1], 0.0)
                if empty_loop:
                    one_rep()
            else:
                for _rep in range(repeat):
                    one_rep()

    nc.compile()
    return nc


def _get_nc():
    global _nc_cache
    if _nc_cache is None:
        _nc_cache = _build()
    return _nc_cache


def _get_runner():
    """Build the sharded jitted executable once."""
    global _run_cache
    if _run_cache is not None:
        return _run_cache

    import jax
    from jax.sharding import Mesh, PartitionSpec
    from jax.experimental.shard_map import shard_map
    import concourse.mybir as mybir
    from concourse.bass2jax import (
        _bass_exec_p, install_neuronx_cc_hook, partition_id_tensor)

    nc = _get_nc()
    install_neuronx_cc_hook()

    partition_name = (nc.partition_id_tensor.name
                      if nc.partition_id_tensor else None)
    in_names, out_names, out_avals = [], [], []
    for alloc in nc.m.functions[0].allocations:
        if not isinstance(alloc, mybir.MemoryLocationSet):
            continue
        name = alloc.memorylocations[0].name
        if alloc.kind == "ExternalInput":
            if name != partition_name:
                in_names.append(name)
        elif alloc.kind == "ExternalOutput":
            out_names.append(name)
            out_avals.append(jax.core.ShapedArray(
                tuple(alloc.tensor_shape), mybir.dt.np(alloc.dtype)))
    n_params = len(in_names)
    n_outs = len(out_avals)
    all_in_names = list(in_names) + list(out_names)
    if partition_name is not None:
        all_in_names.append(partition_name)

    def _body(*args):
        operands = list(args)
        if partition_name is not None:
            operands.append(partition_id_tensor())
        outs = _bass_exec_p.bind(
            *operands,
            out_avals=tuple(out_avals),
            in_names=tuple(all_in_names),
            out_names=tuple(out_names),
            lowering_input_output_aliases=(),
            sim_require_finite=True,
            sim_require_nnan=True,
            nc=nc,
        )
        return tuple(outs)

    devices = jax.devices()[:N_CORES]
    mesh = Mesh(np.asarray(devices), ("core",))
    in_specs = (PartitionSpec("core"),) * (n_params + n_outs)
    out_specs = (PartitionSpec("core"),) * n_outs
    donate = tuple(range(n_params, n_params + n_outs))
    sharded = jax.jit(
        shard_map(_body, mesh=mesh, in_specs=in_specs, out_specs=out_specs,
                  check_rep=False),
        donate_argnums=donate, keep_unused=True)

    zero_shapes = [(N_CORES * a.shape[0], *a.shape[1:]) for a in out_avals]
    zero_dtypes = [a.dtype for a in out_avals]

    def run(concat_inputs):
        zeros = [np.zeros(s, d) for s, d in zip(zero_shapes, zero_dtypes)]
        out_arrs = sharded(*concat_inputs, *zeros)
        return {name: np.asarray(out_arrs[i]) for i, name in
                enumerate(out_names)}

    _run_cache = run
    return run


def kernel(x: np.ndarray) -> np.ndarray:
    assert x.shape == (16, 1, H, W) and x.dtype == np.int32, (x.shape, x.dtype)
    run = _get_runner()
    xs = np.ascontiguousarray(x[:, 0])          # [16, 512, 512]
    out = run([xs])["y_out"]
    # device emits a contiguous per-bank dump; undo the layout permutation
    # (pure reshape/transpose) and prepend the constant zero column
    strip = 8 * G
    nq = 128 // strip
    sg = 512 // strip
    nbank = 2 * L // (sg * nq * G)
    r = out.reshape(N_CORES, nbank, nq, 8, G, sg, 8)  # c,bank,q,h,l,sg,d
    r = r.transpose(0, 1, 5, 2, 4, 3, 6)              # c,bank,sg,q,l,h,d
    r = r.reshape(16, L, NC_CLS)
    full = np.zeros((16, L, NC_CLS + 1), np.float32)
    full[:, :, 1:] = r
    return full


# revision 7
# speedup vs baseline: 1.1508x; 1.0300x over previous
"""BlockWiseHistogramEncoder Trainium2 kernel (digit-factorized PE design).

Input  x: [16, 1, 512, 512] int32, values in [0, 64).
Output:   [16, 1024, 65] float32. out[b, l, 1+v] = count(v in block l)/256,
out[b, l, 0] = 0. Blocks are 16x16, row-major (32x32 grid).

Sharding: pure data parallel over batch - 2 batches per core on 8 cores.

Per-core algorithm (2 batches, 2048 blocks of 256 elems):
  1. DMA-load block tiles [128 blocks, 256 elems] int32, issue split
     across the SP and ACT HWDGE queues.
  2. GPSIMD converts int32 -> bf16 (exact, values < 64).
  3. TensorE transposes tiles to element-major XT[b][eh][e128, l1024]
     (identity matmuls); the DVE PSUM->SBUF copy converts bf16 -> int16.
  4. DVE builds digit masks at 4x: v = 8*hi + lo;
     U_h[e,l] = ((v&56)==8h), V_d[e,l] = ((v&7)==d) as bf16 (16 masks),
     stored group-interleaved so matmul operand slices are single-dim.
  5. TensorE computes per-block joint counts: per group of G=4 blocks,
     out[(h,l),(d,l')] = sum_e U[e,(h,l)] * V[e,(d,l')] via 2 PSUM-
     accumulated 32-col matmuls (e-halves), 4 groups column-tiled per
     PSUM strip via tile_position, 16 supergroups per bank; (eh, q)
     interleaved so LDWEIGHTS pulls ahead of in-flight matmuls.
  6. ScalarE copies each PSUM bank to SBUF fp32 with 1/256 scaling.
  7. DVE extracts the l'==l diagonal with G copy_predicated ops
     (constant per-partition selector masks, p = 32q + 4h + l).
  8. Staging is DMA-dumped contiguously (512B runs); the host undoes
     the pure layout permutation and prepends the constant zero column.
Emission is software-pipelined: batch 1 loads are emitted between batch
0 mask-builds and batch 0 matmul/extraction so ACT-queue DMA issues
overlap compute.

HW-measured (tc.For_i-looped differential timing): ~78 us/core vs the
direct per-class baseline at 267 us/core by the same method.
"""
import sys

if "/opt/trn_rl_repo" not in sys.path:
    sys.path.insert(0, "/opt/trn_rl_repo")

import numpy as np

N_CORES = 8
B_PER_CORE = 2
H = W = 512
NC_CLS = 64
BLK = 16
HB = H // BLK          # 32 blocks per side
L = HB * HB            # 1024 blocks per batch
E = BLK * BLK          # 256 elems per block
TILES = L // 128       # 8 tiles of 128 blocks per batch

G = 4                  # blocks per matmul group (stationary = 8*G cols)

_nc_cache = None
_run_cache = None


def _build(repeat: int = 1, hw_loop: bool = False, empty_loop: bool = False,
           stage: str = "full", g: int = G):
    import concourse.bacc as bacc
    STRIP = 8 * g              # stationary cols per group
    NQ = 128 // STRIP          # col-tiled groups across partitions
    SG = 512 // STRIP          # supergroups (free slots) per PSUM bank
    NBANK = 2 * L // (SG * NQ * g)   # banks per core
    import concourse.mybir as mybir
    import concourse.tile as tile

    fp32 = mybir.dt.float32
    bf16 = mybir.dt.bfloat16
    i32 = mybir.dt.int32
    i16 = mybir.dt.int16

    nc = bacc.Bacc("TRN2", target_bir_lowering=False, debug=False)
    x = nc.dram_tensor("x_in", [B_PER_CORE, H, W], i32, kind="ExternalInput")
    # raw contiguous dump of the extraction staging; host unshuffles the
    # pure layout permutation and prepends the constant zero column
    y = nc.dram_tensor("y_out", [NBANK, 128, SG * 8], fp32,
                       kind="ExternalOutput")

    with tile.TileContext(nc) as tc:
        with tc.tile_pool(name="cst", bufs=1) as c_pool, \
             tc.tile_pool(name="io", bufs=4) as io_pool, \
             tc.tile_pool(name="cv", bufs=4) as cv_pool, \
             tc.tile_pool(name="xt", bufs=1) as xt_pool, \
             tc.tile_pool(name="mk", bufs=1) as mk_pool, \
             tc.tile_pool(name="ex", bufs=3) as ex_pool, \
             tc.tile_pool(name="ptr", bufs=4, space="PSUM") as ptr_pool, \
             tc.tile_pool(name="pmm", bufs=3, space="PSUM") as pmm_pool:

            # ---- constants ----
            # bf16 identity for PE transposes: iota(f - p) == 0
            ident = c_pool.tile([128, 128], bf16)
            iot = c_pool.tile([128, 128], i16, tag="iota")
            nc.gpsimd.iota(iot[:], pattern=[[1, 128]], base=0,
                           channel_multiplier=-1)
            nc.vector.tensor_scalar(
                ident[:], iot[:], 0, None, mybir.AluOpType.is_equal)

            # per-partition selector masks for the diagonal extraction:
            # p = STRIP*q + g*h + l  ->  l(p) = p%g; msk[lam] = [l(p)==lam]
            MW = SG * 8
            pidx = c_pool.tile([128, MW], i32, tag="pidx")
            nc.gpsimd.iota(pidx[:], pattern=[[0, MW]], base=0,
                           channel_multiplier=1)
            lsel = c_pool.tile([128, MW], i32, tag="lsel")
            nc.vector.tensor_scalar(
                lsel[:], pidx[:], g - 1, None, mybir.AluOpType.bitwise_and)
            msk = []
            for lam in range(g):
                m = c_pool.tile([128, MW], i32, tag=f"msk{lam}",
                                name=f"msk{lam}")
                nc.vector.tensor_scalar(
                    m[:], lsel[:], lam, None, mybir.AluOpType.is_equal)
                msk.append(m)

            # persistent per-batch tensors
            XT = [[xt_pool.tile([128, L], i16, tag=f"xt{b}{eh}",
                                name=f"xt{b}{eh}") for eh in range(2)]
                  for b in range(B_PER_CORE)]
            # U interleaved: [e, group, h, l] so per-group weight slices
            # (h-major, l-minor) merge to one contiguous free dim; per-h
            # builds write packed (g, l) slices at 4x.
            U = [[mk_pool.tile([128, L // g, 8, g], bf16, tag=f"u{b}{eh}",
                               name=f"u{b}{eh}") for eh in range(2)]
                 for b in range(B_PER_CORE)]
            # V interleaved: [e, group, d, l'] so per-group moving slices
            # (d-major, l'-minor) merge to one contiguous free dim, while
            # per-d mask builds still write packed (g, l') slices at 4x.
            V = [[mk_pool.tile([128, L // g, 8, g], bf16, tag=f"v{b}{eh}",
                               name=f"v{b}{eh}") for eh in range(2)]
                 for b in range(B_PER_CORE)]

            xbs = [x.ap()[b].rearrange("(bh r) (bw c) -> bh bw r c",
                                       r=BLK, c=BLK)
                   for b in range(B_PER_CORE)]

            def load_tile(b, t):
                # [128 blocks, 256 elems] int32, strided DMA (64B chunks),
                # split across the SP and ACT hwdge queues
                t_in = io_pool.tile([128, E], i32)
                for i in range(4):
                    dst = t_in[32 * i:32 * (i + 1), :].rearrange(
                        "bw (r c) -> bw r c", c=BLK)
                    eng = nc.sync if i % 2 == 0 else nc.scalar
                    eng.dma_start(dst, xbs[b][4 * t + i])
                return t_in

            def convert_tile(t_in):
                # int32 -> bf16 (values < 64, exact)
                tb = cv_pool.tile([128, E], bf16)
                nc.gpsimd.tensor_copy(tb[:], t_in[:])
                return tb

            tr_state = {}

            def transpose_tile(b, t, tb):
                # two 128x128 bf16 transposes -> PSUM; copies PSUM->SBUF
                # (bf16 -> int16 convert) batched across 2 adjacent tiles
                pair = t // 2
                if pair not in tr_state:
                    tr_state[pair] = ptr_pool.tile([128, 512], bf16,
                                                   name="p_tr")
                p_tr = tr_state[pair]
                off = 256 * (t % 2)
                for eh in range(2):
                    nc.tensor.transpose(
                        p_tr[:, off + 128 * eh:off + 128 * (eh + 1)],
                        tb[:, 128 * eh:128 * (eh + 1)], ident[:])
                if off == 256:
                    del tr_state[pair]
                    col0 = 256 * pair
                    pv = p_tr[:].rearrange("e (u eh l) -> e u eh l",
                                           u=2, eh=2)
                    for eh in range(2):
                        nc.vector.tensor_copy(
                            XT[b][eh][:, col0:col0 + 256],
                            pv[:, :, eh, :])

            def build_masks(b, eh):
                xt = XT[b][eh]
                xh = cv_pool.tile([128, L], i16, tag="xh", name="xh")
                nc.vector.tensor_scalar(
                    xh[:], xt[:], 56, None, mybir.AluOpType.bitwise_and)
                xl = cv_pool.tile([128, L], i16, tag="xl", name="xl")
                nc.vector.tensor_scalar(
                    xl[:], xt[:], 7, None, mybir.AluOpType.bitwise_and)
                xhg = xh[:].rearrange("e (g l) -> e g l", l=g)
                xlg = xl[:].rearrange("e (g l) -> e g l", l=g)
                for h in range(8):
                    nc.vector.tensor_scalar(
                        U[b][eh][:, :, h, :], xhg, float(8 * h), None,
                        mybir.AluOpType.is_equal)
                for d in range(8):
                    nc.vector.tensor_scalar(
                        V[b][eh][:, :, d, :], xlg, float(d), None,
                        mybir.AluOpType.is_equal)

            def bank_groups(b, bank):
                # PE: fill one PSUM bank with SG supergroups x NQ col-strips
                pm = pmm_pool.tile([128, 512], fp32)
                # interleave (eh, q) within each sg so consecutive matmuls
                # never target the same col-strip: the PE can pull the next
                # LDWEIGHTS ahead of the in-flight matmul (different col_grp)
                for sg in range(SG):
                    for eh in range(2):
                        for q in range(NQ):
                            gi = (bank * SG + sg) * NQ + q
                            out = pm[STRIP * q:STRIP * (q + 1),
                                     STRIP * sg:STRIP * (sg + 1)]
                            # lhsT: U [e128, (h8, l_g)]; rhs: V [e128, (d8, l'_g)]
                            # out[p=(h,l), f=(d,l')]; diag l'==l is useful
                            nc.tensor.matmul(
                                out, U[b][eh][:, gi], V[b][eh][:, gi],
                                start=(eh == 0), stop=(eh == 1),
                                tile_position=(0, STRIP * q))
                return pm

            def extract_bank(gbank, pm):
                # ScalarE: PSUM fp32 -> SBUF fp32 with 1/256 scaling
                c_sb = ex_pool.tile([128, 512], fp32, tag="c")
                nc.scalar.activation(
                    c_sb[:], pm[:], mybir.ActivationFunctionType.Copy,
                    bias=0.0, scale=1.0 / E)
                # DVE: diagonal extraction -- for each lam, copy f-slice
                # l'=lam into partitions with l(p)==lam (predicated).
                st = ex_pool.tile([128, SG * 8], fp32, tag="st")
                c_v = c_sb[:].rearrange("p (sg d l) -> p sg d l", l=g, d=8)
                for lam in range(g):
                    nc.vector.copy_predicated(
                        st[:], msk[lam][:], c_v[:, :, :, lam])
                # contiguous dump (512B runs); host unshuffles the layout
                nc.sync.dma_start(y.ap()[gbank], st[:])

            # ---------------- pipeline ----------------
            def front(b):
                for t in range(TILES):
                    t_in = load_tile(b, t)
                    if stage == "ld":
                        continue
                    t16 = convert_tile(t_in)
                    if stage == "ldcv":
                        continue
                    transpose_tile(b, t, t16)

            def mid(b):
                if stage in ("ld", "ldcv", "ldtr"):
                    return
                for eh in range(2):
                    build_masks(b, eh)

            def back(b):
                if stage in ("ld", "ldcv", "ldtr", "msk"):
                    return
                nb = NBANK // B_PER_CORE
                for bank in range(nb):
                    pm = bank_groups(b, bank)
                    if stage == "mm":
                        continue
                    extract_bank(b * nb + bank, pm)

            def one_rep():
                # emit b1's loads between b0's masks and b0's banks so the
                # ACT-queue load issues overlap b0 compute, not extraction
                front(0)
                mid(0)
                front(1)
                back(0)
                mid(1)
                back(1)

            if hw_loop:
                with tc.For_i(0, repeat):
                    if not empty_loop:
                        one_rep()
                    else:
                        nc.vector.memset(lsel[:, 0:1], 0.0)
                if empty_loop:
                    one_rep()
            else:
                for _rep in range(repeat):
                    one_rep()

    nc.compile()
    return nc


def _get_nc():
    global _nc_cache
    if _nc_cache is None:
        _nc_cache = _build()
    return _nc_cache


def _get_runner():
    """Build the sharded jitted executable once."""
    global _run_cache
    if _run_cache is not None:
        return _run_cache

    import jax
    from jax.sharding import Mesh, PartitionSpec
    from jax.experimental.shard_map import shard_map
    import concourse.mybir as mybir
    from concourse.bass2jax import (
        _bass_exec_p, install_neuronx_cc_hook, partition_id_tensor)

    nc = _get_nc()
    install_neuronx_cc_hook()

    partition_name = (nc.partition_id_tensor.name
                      if nc.partition_id_tensor else None)
    in_names, out_names, out_avals = [], [], []
    for alloc in nc.m.functions[0].allocations:
        if not isinstance(alloc, mybir.MemoryLocationSet):
            continue
        name = alloc.memorylocations[0].name
        if alloc.kind == "ExternalInput":
            if name != partition_name:
                in_names.append(name)
        elif alloc.kind == "ExternalOutput":
            out_names.append(name)
            out_avals.append(jax.core.ShapedArray(
                tuple(alloc.tensor_shape), mybir.dt.np(alloc.dtype)))
    n_params = len(in_names)
    n_outs = len(out_avals)
    all_in_names = list(in_names) + list(out_names)
    if partition_name is not None:
        all_in_names.append(partition_name)

    def _body(*args):
        operands = list(args)
        if partition_name is not None:
            operands.append(partition_id_tensor())
        outs = _bass_exec_p.bind(
            *operands,
            out_avals=tuple(out_avals),
            in_names=tuple(all_in_names),
            out_names=tuple(out_names),
            lowering_input_output_aliases=(),
            sim_require_finite=True,
            sim_require_nnan=True,
            nc=nc,
        )
        return tuple(outs)

    devices = jax.devices()[:N_CORES]
    mesh = Mesh(np.asarray(devices), ("core",))
    in_specs = (PartitionSpec("core"),) * (n_params + n_outs)
    out_specs = (PartitionSpec("core"),) * n_outs
    donate = tuple(range(n_params, n_params + n_outs))
    sharded = jax.jit(
        shard_map(_body, mesh=mesh, in_specs=in_specs, out_specs=out_specs,
                  check_rep=False),
        donate_argnums=donate, keep_unused=True)

    zero_shapes = [(N_CORES * a.shape[0], *a.shape[1:]) for a in out_avals]
    zero_dtypes = [a.dtype for a in out_avals]

    def run(concat_inputs):
        zeros = [np.zeros(s, d) for s, d in zip(zero_shapes, zero_dtypes)]
        out_arrs = sharded(*concat_inputs, *zeros)
        return {name: np.asarray(out_arrs[i]) for i, name in
                enumerate(out_names)}

    _run_cache = run
    return run


def kernel(x: np.ndarray) -> np.ndarray:
    assert x.shape == (16, 1, H, W) and x.dtype == np.int32, (x.shape, x.dtype)
    run = _get_runner()
    xs = np.ascontiguousarray(x[:, 0])          # [16, 512, 512]
    out = run([xs])["y_out"]
    # device emits a contiguous per-bank dump; undo the layout permutation
    # (pure reshape/transpose) and prepend the constant zero column
    strip = 8 * G
    nq = 128 // strip
    sg = 512 // strip
    nbank = 2 * L // (sg * nq * G)
    r = out.reshape(N_CORES, nbank, nq, 8, G, sg, 8)  # c,bank,q,h,l,sg,d
    r = r.transpose(0, 1, 5, 2, 4, 3, 6)              # c,bank,sg,q,l,h,d
    r = r.reshape(16, L, NC_CLS)
    full = np.zeros((16, L, NC_CLS + 1), np.float32)
    full[:, :, 1:] = r
    return full
